# revision 1
# baseline (speedup 1.0000x reference)
"""Causal self-attention (B=4, T=2048, D=1024, H=16, HD=64) on 8 TRN2 NeuronCores.

Sharding: core = (batch b in 0..3, head-group g in 0..1) -> data parallel on B,
tensor parallel over heads (8 heads per core). Each core computes a partial
out-projection for its head group; the host sums the pair of partials per batch
(the TP all-reduce done at unshard time).

Device kernel (per core), all matmuls in float32r (full PE rate, TF32-ish):
  1. xT = x.T via PE transposes                      [1024, 2048]
  2. v  = x @ Wv_sl (natural orientation) + ones col [2048, 8*65]
     qT/kT = (x @ Wq/Wk_sl).T                        [512, 2048] each
  3. per (head, tq-chunk 512): S.T tiles = kT.T qT; exp(S/8 + causal mask)
     on ACT; AV+Z fused: psum[65,512] = [v_h | 1].T @ P.T accumulated over tk
     (row 64 = softmax denominator Z); normalize via e0-pattern broadcast
     matmul of 1/Z and a DVE multiply -> yT                       [512, 2048]
  4. out = yT.T @ Wout_sl + bout (natural orientation, no final transpose)
Causality: strictly-upper tk-tiles are skipped entirely (half the attention
FLOPs); diagonal tiles get the transposed mask block (from the attn_mask
input, PE-transposed once) added before exp.
"""

import os
import numpy as np

import concourse.bass as bass
import concourse.tile as tile
from concourse import bacc, mybir
import concourse.bass_utils as bass_utils
from concourse.masks import make_identity

F32 = mybir.dt.float32
F32R = mybir.dt.float32r
AF = mybir.ActivationFunctionType
ALU = mybir.AluOpType

B, T, D, H = 4, 2048, 1024, 16
HD = D // H          # 64
G = 2                # head groups (TP degree)
HPG = H // G         # 8 heads per core
DG = HPG * HD        # 512 local qkv dims per core
NT = T // 128        # 16 row tiles
ND = D // 128        # 8 contraction tiles
NC = T // 512        # 4 tq chunks
NK = DG // 128       # 4 local-dim tiles (out-proj contraction)
VW = HD + 1          # 65: v columns per head incl. ones column

_cached = {}


def _build():
    nc = bacc.Bacc("TRN2", target_bir_lowering=False, debug=False, num_devices=8)

    x_d = nc.dram_tensor("x", [T, D], F32R, kind="ExternalInput")
    wq_d = nc.dram_tensor("wq", [D, DG], F32R, kind="ExternalInput")
    wk_d = nc.dram_tensor("wk", [D, DG], F32R, kind="ExternalInput")
    wv_d = nc.dram_tensor("wv", [D, DG], F32R, kind="ExternalInput")
    wo_d = nc.dram_tensor("wo", [DG, D], F32R, kind="ExternalInput")
    bq_d = nc.dram_tensor("bq", [DG], F32, kind="ExternalInput")
    bk_d = nc.dram_tensor("bk", [DG], F32, kind="ExternalInput")
    bv_d = nc.dram_tensor("bv", [DG], F32R, kind="ExternalInput")
    bo_d = nc.dram_tensor("bo", [D], F32R, kind="ExternalInput")
    mask_d = nc.dram_tensor("mask", [512, 512], F32, kind="ExternalInput")
    out_d = nc.dram_tensor("out", [T, D], F32, kind="ExternalOutput")

    with tile.TileContext(nc) as tc:
        with nc.allow_low_precision(reason="fp32r matmul pipeline, fp32 psum"):
            _emit(nc, tc, x_d, wq_d, wk_d, wv_d, wo_d, bq_d, bk_d, bv_d, bo_d,
                  mask_d, out_d)
    nc.finalize()
    return nc


def _emit(nc, tc, x_d, wq_d, wk_d, wv_d, wo_d, bq_d, bk_d, bv_d, bo_d,
          mask_d, out_d):
    from contextlib import ExitStack
    ctx = ExitStack()
    with ctx:
        # ---------------- persistent pools ----------------
        const_p = ctx.enter_context(tc.tile_pool(name="const", bufs=1))
        qk_p = ctx.enter_context(tc.tile_pool(name="qk", bufs=1))
        vp_p = ctx.enter_context(tc.tile_pool(name="vp", bufs=1))
        maskt_p = ctx.enter_context(tc.tile_pool(name="maskt", bufs=1))

        # identities (f32 for mask transpose, f32r for x transpose)
        ident32 = const_p.tile([128, 128], F32, tag="ident32")
        make_identity(nc, ident32[:])
        identr = const_p.tile([128, 128], F32R, tag="identr")
        nc.vector.tensor_copy(identr[:], ident32[:])

        # e0 pattern [128, 128]: row 0 ones, else zeros (broadcast matmuls)
        e0 = const_p.tile([128, 128], F32R, tag="e0")
        nc.vector.memset(e0[:].bitcast(F32), 0.0)
        nc.vector.memset(e0[0:1, :].bitcast(F32), 1.0)

        # q/k bias columns [128, 8]: cols 0-3 = bq tiles, 4-7 = bk tiles
        bqk = const_p.tile([128, 8], F32, tag="bqk")
        nc.sync.dma_start(bqk[:, 0:NK], bq_d[:].rearrange("(f p) -> p f", p=128))
        nc.sync.dma_start(bqk[:, NK:2 * NK], bk_d[:].rearrange("(f p) -> p f", p=128))

        # broadcast bv -> [128, 512] and bout -> [128, 1024] via e0 matmuls
        bvrow = const_p.tile([128, DG], F32R, tag="bvrow")
        nc.vector.memset(bvrow[:].bitcast(F32), 0.0)
        nc.sync.dma_start(bvrow[0:1, :], bv_d[:].rearrange("(o n) -> o n", o=1))
        borow = const_p.tile([128, D], F32R, tag="borow")
        nc.vector.memset(borow[:].bitcast(F32), 0.0)
        nc.sync.dma_start(borow[0:1, :], bo_d[:].rearrange("(o n) -> o n", o=1))
        bv_bc = const_p.tile([128, DG], F32, tag="bv_bc")
        bo_bc = const_p.tile([128, D], F32, tag="bo_bc")
        with tc.tile_pool(name="bc_ps", bufs=3, space="PSUM") as bc_ps:
            pb = bc_ps.tile([128, DG], F32)
            nc.tensor.matmul(pb[:], e0[:], bvrow[:], start=True, stop=True)
            nc.vector.tensor_copy(bv_bc[:], pb[:])
            for oc in range(2):
                po = bc_ps.tile([128, 512], F32, tag="bo")
                nc.tensor.matmul(po[:], e0[:], borow[:, 512 * oc:512 * (oc + 1)],
                                 start=True, stop=True)
                nc.vector.tensor_copy(bo_bc[:, 512 * oc:512 * (oc + 1)], po[:])

        # transposed causal-diagonal mask maskT[l][tk 128, tq 512]
        maskt = [maskt_p.tile([128, 512], F32, tag=f"mt{l}", name=f"maskt{l}") for l in range(4)]
        with (
            tc.tile_pool(name="mload", bufs=1) as mload,
            tc.tile_pool(name="m_ps", bufs=4, space="PSUM") as m_ps,
        ):
            mrows = []
            for i in range(4):
                mt = mload.tile([128, 512], F32, tag=f"mr{i}", name=f"mr{i}")
                nc.sync.dma_start(mt[:], mask_d[128 * i:128 * (i + 1), :])
                mrows.append(mt)
            for l in range(4):
                for i in range(4):
                    tp = m_ps.tile([128, 128], F32, tag="mtp")
                    nc.tensor.transpose(tp[:], mrows[i][:, 128 * l:128 * (l + 1)],
                                        ident32[:])
                    nc.vector.tensor_copy(maskt[l][:, 128 * i:128 * (i + 1)], tp[:])

        # v' tiles [128, 8*65] f32r, ones pre-set (col 64 of each head block)
        vp = []
        for t in range(NT):
            vt = vp_p.tile([128, HPG * VW], F32R, tag=f"vp{t}", name=f"vp{t}")
            nc.vector.memset(vt[:].bitcast(F32), 1.0)
            vp.append(vt)

        # qkT tiles [128, 2048] f32r: 0-3 = qT (head pairs), 4-7 = kT
        qk = [qk_p.tile([128, T], F32R, tag=f"qk{f}", name=f"qk{f}") for f in range(2 * NK)]

        # ---------------- phase 1+2: xT, then v / qT / kT ----------------
        with tc.tile_pool(name="xt", bufs=1) as xt_p:
            xt = [xt_p.tile([128, T], F32R, tag=f"xt{d}", name=f"xt{d}") for d in range(ND)]
            with (
                tc.tile_pool(name="xtmp", bufs=3) as xtmp_p,
                tc.tile_pool(name="t_ps", bufs=4, space="PSUM") as t_ps,
            ):
                for t in range(NT):
                    xtmp = xtmp_p.tile([128, D], F32R, tag="xtmp")
                    nc.sync.dma_start(xtmp[:], x_d[128 * t:128 * (t + 1), :])
                    for d in range(ND):
                        tp = t_ps.tile([128, 128], F32R, tag="xtp")
                        nc.tensor.transpose(
                            tp[:], xtmp[:, 128 * d:128 * (d + 1)], identr[:])
                        nc.vector.tensor_copy(xt[d][:, 128 * t:128 * (t + 1)],
                                              tp[:].bitcast(F32))

            # v projection (natural orientation) + bias + interleave ones
            with (
                tc.tile_pool(name="wv", bufs=1) as wv_p,
                tc.tile_pool(name="v_ps", bufs=4, space="PSUM") as v_ps,
            ):
                wv_sb = []
                for d in range(ND):
                    wt = wv_p.tile([128, DG], F32R, tag=f"wv{d}", name=f"wvt{d}")
                    nc.sync.dma_start(wt[:], wv_d[128 * d:128 * (d + 1), :])
                    wv_sb.append(wt)
                for t in range(NT):
                    pv = v_ps.tile([128, DG], F32, tag="pv")
                    for d in range(ND):
                        nc.tensor.matmul(pv[:], xt[d][:, 128 * t:128 * (t + 1)],
                                         wv_sb[d][:], start=(d == 0),
                                         stop=(d == ND - 1))
                    nc.vector.tensor_tensor(
                        vp[t][:].rearrange("p (h c) -> p h c", h=HPG)[:, :, 0:HD],
                        pv[:].rearrange("p (h c) -> p h c", h=HPG),
                        bv_bc[:].rearrange("p (h c) -> p h c", h=HPG),
                        ALU.add)

            # q/k projections (transposed orientation)
            with (
                tc.tile_pool(name="wblk", bufs=2) as wblk_p,
                tc.tile_pool(name="qk_ps", bufs=4, space="PSUM") as qk_ps,
            ):
                # emit in q0,k0,q1,k1,... order so attention head pairs
                # unblock as early as possible
                for f in [0, NK, 1, NK + 1, 2, NK + 2, 3, NK + 3]:
                    src = wq_d if f < NK else wk_d
                    fc = f % NK
                    wblk = wblk_p.tile([128, D], F32R, tag="wblk")
                    nc.sync.dma_start(
                        wblk[:].rearrange("p (dt c) -> p dt c", dt=ND),
                        src[:, 128 * fc:128 * (fc + 1)].rearrange(
                            "(dt p) c -> p dt c", p=128))
                    for c in range(NC):
                        pq = qk_ps.tile([128, 512], F32, tag="pq")
                        for d in range(ND):
                            nc.tensor.matmul(
                                pq[:], wblk[:, 128 * d:128 * (d + 1)],
                                xt[d][:, 512 * c:512 * (c + 1)],
                                start=(d == 0), stop=(d == ND - 1))
                        nc.vector.tensor_scalar(
                            qk[f][:, 512 * c:512 * (c + 1)], pq[:],
                            bqk[:, f:f + 1], None, ALU.add)

        # ---------------- phase 3: attention ----------------
        with tc.tile_pool(name="yt", bufs=1) as yt_p:
          yt = [yt_p.tile([128, T], F32R, tag=f"yt{k}", name=f"yt{k}") for k in range(NK)]
          # Z rows collected at partition i = h*NC + c; reciprocal'd in one op
          # after attention so no slow reciprocal sits on the attention path.
          zmat = yt_p.tile([128, 512], F32, tag="zmat", name="zmat")
          zrec = yt_p.tile([128, 512], F32R, tag="zrec", name="zrec")
          nc.vector.memset(zmat[:], 1.0)
          nc.vector.memset(zrec[:].bitcast(F32), 0.0)
          with (
            tc.tile_pool(name="pp", bufs=4) as pp_p,
            tc.tile_pool(name="stmp", bufs=3) as stmp_p,
            tc.tile_pool(name="ztmp", bufs=3) as ztmp_p,
            tc.tile_pool(name="s_ps", bufs=3, space="PSUM") as s_ps,
            tc.tile_pool(name="y_ps", bufs=3, space="PSUM") as y_ps,
          ):
            for h in range(HPG):
                qrow = (h % 2) * 64
                qt = qk[h // 2]
                kt = qk[NK + h // 2]
                for c in range(NC):
                    jmax = 4 * c + 3
                    py = y_ps.tile([VW, 512], F32, tag="py")
                    for j in range(jmax + 1):
                        ps = s_ps.tile([128, 512], F32, tag="ps")
                        nc.tensor.matmul(
                            ps[:],
                            kt[qrow:qrow + HD, 128 * j:128 * (j + 1)],
                            qt[qrow:qrow + HD, 512 * c:512 * (c + 1)],
                            start=True, stop=True)
                        pt = pp_p.tile([128, 512], F32R, tag="pt")
                        if j >= 4 * c:  # diagonal block: add transposed mask
                            st = stmp_p.tile([128, 512], F32, tag="st")
                            nc.vector.tensor_tensor(st[:], ps[:],
                                                    maskt[j - 4 * c][:], ALU.add)
                            nc.scalar.activation(pt[:], st[:], AF.Exp, scale=0.125)
                        else:
                            nc.scalar.activation(pt[:], ps[:], AF.Exp, scale=0.125)
                        nc.tensor.matmul(
                            py[:], vp[j][:, VW * h:VW * (h + 1)], pt[:],
                            start=(j == 0), stop=(j == jmax),
                            skip_group_check=True)
                    # cheap psum drain: raw yT + Z row (normalized later)
                    i = h * NC + c
                    nc.vector.tensor_copy(
                        yt[h // 2][qrow:qrow + HD, 512 * c:512 * (c + 1)],
                        py[0:64, :])
                    zt = ztmp_p.tile([1, 512], F32, tag="zt")
                    nc.vector.tensor_copy(zt[:], py[64:65, :])
                    nc.sync.dma_start(zmat[i:i + 1, :], zt[:])

          # ---------------- normalization pass ----------------
          with (
            tc.tile_pool(name="escr", bufs=1) as escr_p,
            tc.tile_pool(name="zbc", bufs=3) as zbc_p,
            tc.tile_pool(name="n_ps", bufs=3, space="PSUM") as n_ps,
          ):
            # E matrix: 32 stacked e_i patterns; E[p, 64*i + m] = (p == i)
            e32 = escr_p.tile([128, HPG * NC * 64], F32, tag="e32")
            nc.gpsimd.memset(e32[:], 1.0)
            nc.gpsimd.affine_select(
                out=e32[:], in_=e32[:], compare_op=ALU.is_equal, fill=0.0,
                base=0, channel_multiplier=1, pattern=[[-1, HPG * NC], [0, 64]])
            emat = escr_p.tile([128, HPG * NC * 64], F32R, tag="emat")
            nc.vector.tensor_copy(emat[:], e32[:])
            nc.vector.reciprocal(zrec[0:HPG * NC, :], zmat[0:HPG * NC, :])
            for c in range(NC):
                for h in range(HPG):
                    i = h * NC + c
                    qrow = (h % 2) * 64
                    pz = n_ps.tile([64, 512], F32, tag="pz")
                    nc.tensor.matmul(pz[:], emat[:, 64 * i:64 * (i + 1)],
                                     zrec[:], start=True, stop=True)
                    zbc = zbc_p.tile([128, 512], F32, tag="zbc")
                    nc.vector.tensor_copy(zbc[qrow:qrow + HD, :], pz[:])
                    ysl = yt[h // 2][qrow:qrow + HD, 512 * c:512 * (c + 1)]
                    nc.vector.tensor_tensor(ysl, ysl, zbc[qrow:qrow + HD, :],
                                            ALU.mult)

          # ---------------- phase 4: out projection ----------------
          with (
                tc.tile_pool(name="wo", bufs=1) as wo_p,
                tc.tile_pool(name="osb", bufs=3) as osb_p,
                tc.tile_pool(name="o_ps", bufs=4, space="PSUM") as o_ps,
            ):
                wo_sb = []
                for k in range(NK):
                    wt = wo_p.tile([128, D], F32R, tag=f"wo{k}", name=f"wot{k}")
                    nc.sync.dma_start(wt[:], wo_d[128 * k:128 * (k + 1), :])
                    wo_sb.append(wt)
                for t in range(NT):
                    for oc in range(2):
                        po = o_ps.tile([128, 512], F32, tag="po")
                        for k in range(NK):
                            nc.tensor.matmul(
                                po[:], yt[k][:, 128 * t:128 * (t + 1)],
                                wo_sb[k][:, 512 * oc:512 * (oc + 1)],
                                start=(k == 0), stop=(k == NK - 1))
                        ob = osb_p.tile([128, 512], F32, tag="ob")
                        nc.vector.tensor_tensor(
                            ob[:], po[:], bo_bc[:, 512 * oc:512 * (oc + 1)],
                            ALU.add)
                        nc.sync.dma_start(
                            out_d[128 * t:128 * (t + 1),
                                  512 * oc:512 * (oc + 1)], ob[:])


def kernel(x, attn_mask, Wqkv, bqkv, Wout, bout):
    if "nc" not in _cached:
        _cached["nc"] = _build()
    nc = _cached["nc"]

    x = np.ascontiguousarray(x, dtype=np.float32)
    Wqkv = np.ascontiguousarray(Wqkv, dtype=np.float32)
    bqkv = np.ascontiguousarray(bqkv, dtype=np.float32)
    Wout = np.ascontiguousarray(Wout, dtype=np.float32)
    bout = np.ascontiguousarray(bout, dtype=np.float32)
    mask_blk = np.ascontiguousarray(attn_mask[0, 0, :512, :512], dtype=np.float32)

    zeros_bo = np.zeros_like(bout)
    in_maps = []
    for b in range(B):
        for g in range(G):
            s = slice(g * DG, (g + 1) * DG)
            in_maps.append({
                "x": np.ascontiguousarray(x[b]),
                "wq": np.ascontiguousarray(Wqkv[:, g * DG:(g + 1) * DG]),
                "wk": np.ascontiguousarray(Wqkv[:, D + g * DG:D + (g + 1) * DG]),
                "wv": np.ascontiguousarray(Wqkv[:, 2 * D + g * DG:2 * D + (g + 1) * DG]),
                "wo": np.ascontiguousarray(Wout[s, :]),
                "bq": np.ascontiguousarray(bqkv[g * DG:(g + 1) * DG]),
                "bk": np.ascontiguousarray(bqkv[D + g * DG:D + (g + 1) * DG]),
                "bv": np.ascontiguousarray(bqkv[2 * D + g * DG:2 * D + (g + 1) * DG]),
                "bo": bout if g == 0 else zeros_bo,
                "mask": mask_blk,
            })

    trace = bool(int(os.environ.get("BASS_ATTN_TRACE", "0")))
    res = bass_utils.run_bass_kernel_spmd(
        nc, in_maps, core_ids=list(range(B * G)), trace=trace)
    _cached["last_result"] = res

    out = np.empty((B, T, D), dtype=np.float32)
    for b in range(B):
        out[b] = res.results[2 * b]["out"] + res.results[2 * b + 1]["out"]
    return out



# revision 13
# speedup vs baseline: 1.5496x; 1.5496x over previous
"""Causal self-attention (B=4, T=2048, D=1024, H=16, HD=64) on 8 TRN2 NeuronCores.

Sharding: core = (batch b in 0..3, head-group g in 0..1) -> data parallel on B,
tensor parallel over heads (8 heads per core). Each core computes a partial
out-projection for its head group; the host sums the pair of partials per batch
(plus bout) at unshard time.

Device kernel (per core), bf16 matmul pipeline with fp32 PSUM accumulate:
  1. xT via DMA-transpose (bf16 XBAR path)                     [128 x 2048] x8
  2. v  = x @ Wv (natural) + ones col interleaved per head     [2048, 8*65]
     qT/kT = (x @ Wq/Wk).T per head-pair, bias added on drain  [128, 2048] x8
  3. attention, software-pipelined in groups of 2 tk-tiles:
       S^T pair -> PSUM [128,1024] (mask accumulated via PE on diag tiles,
       strictly-upper columns trimmed), one batched EXP (ACT) per group into
       bf16, AV+Z fused matmuls (M=65, ones row = softmax denominator).
     ACT exp is the bottleneck engine; qk projections of the next head-pair
     and the 1/Z broadcast of the previous pair are chopped into small PE
     pieces and emitted between S(g) and AV(g-1) so ACT never starves.
  4. normalization per pair: batched reciprocal + esel-matmul broadcast of
     1/Z, yt scaled in place.
  5. out = yT.T @ Wout streamed PSUM -> DRAM (bout added on host).
"""

import os
import numpy as np
from ml_dtypes import bfloat16

import concourse.bass as bass
import concourse.tile as tile
from concourse import bacc, mybir
import concourse.bass_utils as bass_utils
from concourse.masks import make_identity

F32 = mybir.dt.float32
F32R = mybir.dt.float32r
BF16 = mybir.dt.bfloat16
AF = mybir.ActivationFunctionType
ALU = mybir.AluOpType

B, T, D, H = 4, 2048, 1024, 16
HD = D // H          # 64
G = 2                # head groups (TP degree)
HPG = H // G         # 8 heads per core
DG = HPG * HD        # 512 local qkv dims per core
NT = T // 128        # 16 row tiles
ND = D // 128        # 8 contraction tiles
NC = T // 512        # 4 tq chunks
NK = DG // 128       # 4 local-dim tiles (pairs)
VW = HD + 1          # 65: v columns per head incl. ones column

_cached = {}


def _build():
    nc = bacc.Bacc("TRN2", target_bir_lowering=False, debug=False, num_devices=8)

    x_d = nc.dram_tensor("x", [T, D], BF16, kind="ExternalInput")
    wq_d = nc.dram_tensor("wq", [D, DG], BF16, kind="ExternalInput")
    wk_d = nc.dram_tensor("wk", [D, DG], BF16, kind="ExternalInput")
    wv_d = nc.dram_tensor("wv", [D, DG], BF16, kind="ExternalInput")
    wo_d = nc.dram_tensor("wo", [DG, D], BF16, kind="ExternalInput")
    bq_d = nc.dram_tensor("bq", [DG], F32, kind="ExternalInput")
    bk_d = nc.dram_tensor("bk", [DG], F32, kind="ExternalInput")
    bv_d = nc.dram_tensor("bv", [DG], BF16, kind="ExternalInput")
    maskt_d = nc.dram_tensor("maskt", [512, 512], BF16, kind="ExternalInput")
    out_d = nc.dram_tensor("out", [T, D], F32, kind="ExternalOutput")

    with tile.TileContext(nc) as tc:
        with nc.allow_low_precision(reason="bf16 matmul pipeline, fp32 psum"):
            _emit(nc, tc, x_d, wq_d, wk_d, wv_d, wo_d, bq_d, bk_d, bv_d,
                  maskt_d, out_d)
    nc.finalize()
    return nc


def _emit(nc, tc, x_d, wq_d, wk_d, wv_d, wo_d, bq_d, bk_d, bv_d, maskt_d,
          out_d):
    from contextlib import ExitStack
    ctx = ExitStack()
    with ctx:
        # ---------------- pools ----------------
        const_p = ctx.enter_context(tc.tile_pool(name="const", bufs=1))
        xt_p = ctx.enter_context(tc.tile_pool(name="xt", bufs=1))
        qk_p = ctx.enter_context(tc.tile_pool(name="qk", bufs=1))
        vp_p = ctx.enter_context(tc.tile_pool(name="vp", bufs=1))
        yt_p = ctx.enter_context(tc.tile_pool(name="yt", bufs=1))
        w_p = ctx.enter_context(tc.tile_pool(name="w", bufs=1))
        wblk_p = ctx.enter_context(tc.tile_pool(name="wblk", bufs=4))
        pt_p = ctx.enter_context(tc.tile_pool(name="pt", bufs=3))
        zt_p = ctx.enter_context(tc.tile_pool(name="zt", bufs=3))
        ob_p = ctx.enter_context(tc.tile_pool(name="ob", bufs=3))
        s_ps = ctx.enter_context(tc.tile_pool(name="s_ps", bufs=3, space="PSUM"))
        y_ps = ctx.enter_context(tc.tile_pool(name="y_ps", bufs=2, space="PSUM"))

        def sps():
            return s_ps.tile([128, 1024], F32, tag="s", name="sgrp")

        # ---------------- constants ----------------
        identb = const_p.tile([128, 128], BF16, tag="identb")
        make_identity(nc, identb[:])

        # e0b: row 0 ones (bf16, bias broadcast)
        e0b = const_p.tile([128, 128], BF16, tag="e0b")
        nc.vector.memset(e0b[:], 0.0)
        nc.vector.memset(e0b[0:1, :], 1.0)

        # esel[p, 64*i + m] = (p == i) for i in 0..7 (1/Z broadcast select)
        e32 = const_p.tile([128, 512], F32, tag="e32")
        nc.gpsimd.memset(e32[:], 1.0)
        nc.gpsimd.affine_select(
            out=e32[:], in_=e32[:], compare_op=ALU.is_equal, fill=0.0,
            base=0, channel_multiplier=1, pattern=[[-1, 8], [0, 64]])
        esel = const_p.tile([128, 512], F32R, tag="esel")
        nc.vector.tensor_copy(esel[:], e32[:])

        # q/k bias columns [128, 8]: cols 0-3 = bq tiles, 4-7 = bk tiles
        bqk = const_p.tile([128, 8], F32, tag="bqk")
        nc.sync.dma_start(bqk[:, 0:NK], bq_d[:].rearrange("(f p) -> p f", p=128))
        nc.sync.dma_start(bqk[:, NK:2 * NK], bk_d[:].rearrange("(f p) -> p f", p=128))

        # bv broadcast [128, 512]
        bvrow = const_p.tile([128, DG], BF16, tag="bvrow")
        nc.vector.memset(bvrow[:], 0.0)
        nc.sync.dma_start(bvrow[0:1, :], bv_d[:].rearrange("(o n) -> o n", o=1))
        bv_bc = const_p.tile([128, DG], BF16, tag="bv_bc")
        pb = sps()
        nc.tensor.matmul(pb[:, 0:DG], e0b[:], bvrow[:], start=True, stop=True)
        nc.vector.tensor_copy(bv_bc[:], pb[:, 0:DG])

        # zmat (Z rows, one tile per pair) / zrec (1/Z, rows 0-7 live)
        zmat = [const_p.tile([8, 512], F32, tag=f"zmat{p}", name=f"zmat{p}")
                for p in range(NK)]
        zrec = const_p.tile([128, 512], F32R, tag="zrec")
        nc.vector.memset(zrec[:].bitcast(F32), 0.0)

        # transposed causal diag mask tiles (host pre-transposed)
        maskt = []
        for l in range(4):
            mt = const_p.tile([128, 512], BF16, tag=f"mt{l}", name=f"maskt{l}")
            nc.sync.dma_start(mt[:], maskt_d[128 * l:128 * (l + 1), :])
            maskt.append(mt)

        # ---------------- xT via DMA transpose (chunked) ----------------
        xt = [xt_p.tile([128, T], BF16, tag=f"xt{d}", name=f"xt{d}") for d in range(ND)]
        for cc in range(NC):
            for d in range(ND):
                nc.sync.dma_start_transpose(
                    xt[d][:, 512 * cc:512 * (cc + 1)],
                    x_d[512 * cc:512 * (cc + 1), 128 * d:128 * (d + 1)])

        # ---------------- weights ----------------
        wv_sb = []
        for d in range(ND):
            wt = w_p.tile([128, DG], BF16, tag=f"wv{d}", name=f"wvt{d}")
            nc.sync.dma_start(wt[:], wv_d[128 * d:128 * (d + 1), :])
            wv_sb.append(wt)
        wo_sb = []
        for k in range(NK):
            wt = w_p.tile([128, D], BF16, tag=f"wo{k}", name=f"wot{k}")
            nc.sync.dma_start(wt[:], wo_d[128 * k:128 * (k + 1), :])
            wo_sb.append(wt)

        def load_wblk(is_k, f):
            src = wk_d if is_k else wq_d
            wblk = wblk_p.tile([128, D], BF16, tag="wblk")
            nc.sync.dma_start(
                wblk[:].rearrange("p (dt c) -> p dt c", dt=ND),
                src[:, 128 * f:128 * (f + 1)].rearrange("(dt p) c -> p dt c", p=128))
            return wblk

        # persistent SBUF tensors
        vp = []
        for t in range(NT):
            vt = vp_p.tile([128, HPG * VW], BF16, tag=f"vp{t}", name=f"vp{t}")
            nc.vector.memset(vt[:], 1.0)
            vp.append(vt)
        qk = [qk_p.tile([128, T], BF16, tag=f"qk{f}", name=f"qk{f}")
              for f in range(2 * NK)]
        yt = [yt_p.tile([128, T], BF16, tag=f"yt{k}", name=f"yt{k}")
              for k in range(NK)]

        # ---------------- v projection (plain, upfront) ----------------
        for t in range(NT):
            pv = sps()
            for d in range(ND):
                nc.tensor.matmul(pv[:, 0:DG], xt[d][:, 128 * t:128 * (t + 1)],
                                 wv_sb[d][:], start=(d == 0), stop=(d == ND - 1))
            nc.vector.tensor_tensor(
                vp[t][:].rearrange("p (h c) -> p h c", h=HPG)[:, :, 0:HD],
                pv[:, 0:DG].rearrange("p (h c) -> p h c", h=HPG),
                bv_bc[:].rearrange("p (h c) -> p h c", h=HPG),
                ALU.add)

        # qk projection piece generator: one f-block = 4 chunks x 8 d-matmuls,
        # yielded in 8 pieces of 4 matmuls (chunk halves), drain per chunk.
        # The psum tile is allocated when the first half RUNS (not at
        # generator-build time) so pool rotation order matches emission order.
        def qk_pieces(is_k, f):
            wblk = load_wblk(is_k, f)
            dst = qk[NK + f if is_k else f]
            bcol = NK + f if is_k else f
            state = {}
            for cidx in range(NC):
                for half in range(2):
                    def piece(wblk=wblk, cidx=cidx, half=half,
                              dst=dst, bcol=bcol):
                        if half == 0:
                            state[cidx] = sps()
                        pq = state[cidx]
                        for d in range(4 * half, 4 * half + 4):
                            nc.tensor.matmul(
                                pq[:, 0:512], wblk[:, 128 * d:128 * (d + 1)],
                                xt[d][:, 512 * cidx:512 * (cidx + 1)],
                                start=(d == 0), stop=(d == ND - 1))
                        if half == 1:
                            nc.vector.tensor_scalar(
                                dst[:, 512 * cidx:512 * (cidx + 1)],
                                pq[:, 0:512], bqk[:, bcol:bcol + 1], None,
                                ALU.add)
                    yield piece

        # qk pair 0 upfront (plain)
        for is_k in (False, True):
            for pc in qk_pieces(is_k, 0):
                pc()

        # ---------------- attention ----------------
        # per-pair normalization pieces (reciprocal batched per pair,
        # broadcast via esel matmul, yt scaled in place)
        def norm_pieces(p):
            def recip(p=p):
                nc.vector.reciprocal(zrec[0:8, :], zmat[p][0:8, :])
            yield recip
            for hl in range(2):
                for c in range(NC):
                    def piece(p=p, hl=hl, c=c):
                        i = 4 * hl + c
                        qrow = 64 * hl
                        zb = sps()
                        nc.tensor.matmul(zb[0:64, 0:512],
                                         esel[:, 64 * i:64 * (i + 1)],
                                         zrec[:],
                                         start=True, stop=True)
                        ysl = yt[p][qrow:qrow + HD, 512 * c:512 * (c + 1)]
                        nc.vector.tensor_tensor(ysl, ysl, zb[0:64, 0:512],
                                                ALU.mult)
                    yield piece

        filler_q = []  # queued PE filler pieces

        class Grp:
            __slots__ = ("h", "c", "j0", "ps", "pt", "py", "chunk_end",
                         "pair_end")

        def make_groups(p):
            gs = []
            for hl in range(2):
                for c in range(NC):
                    for j0 in range(0, 4 * c + 4, 2):
                        g = Grp()
                        g.h, g.c, g.j0 = 2 * p + hl, c, j0
                        g.chunk_end = (j0 + 1 == 4 * c + 3)
                        g.pair_end = g.chunk_end and hl == 1 and c == NC - 1
                        gs.append(g)
            return gs

        def emit_S(g):
            p = g.h // 2
            qrow = 64 * (g.h % 2)
            qt, kt = qk[p], qk[NK + p]
            g.ps = sps()
            for idx in range(2):
                j = g.j0 + idx
                d = j - 4 * g.c
                off = 128 * d if d >= 0 else 0
                col0 = 512 * idx + off
                nc.tensor.matmul(
                    g.ps[:, col0:512 * (idx + 1)],
                    kt[qrow:qrow + HD, 128 * j:128 * (j + 1)],
                    qt[qrow:qrow + HD, 512 * g.c + off:512 * (g.c + 1)],
                    start=True, stop=(d < 0), skip_group_check=True)
                if d >= 0:
                    nc.tensor.matmul(
                        g.ps[:, col0:512 * (idx + 1)], identb[:],
                        maskt[d][:, off:512],
                        start=False, stop=True, skip_group_check=True)

        def emit_EXP(g):
            g.pt = pt_p.tile([128, 1024], BF16, tag="pt", name="pt")
            nc.scalar.activation(g.pt[:], g.ps[:], AF.Exp, scale=0.125)

        def emit_AV(g, py_live):
            hidx = g.h % HPG
            if g.j0 == 0:
                py_live[0] = y_ps.tile([VW, 512], F32, tag="py", name="py")
            g.py = py_live[0]
            jmax = 4 * g.c + 3
            for idx in range(2):
                j = g.j0 + idx
                d = j - 4 * g.c
                off = 128 * d if d >= 0 else 0
                nc.tensor.matmul(
                    g.py[:, off:512],
                    vp[j][:, VW * hidx:VW * hidx + VW],
                    g.pt[:, 512 * idx + off:512 * (idx + 1)],
                    start=(j == 0), stop=(j == jmax), skip_group_check=True)

        def emit_post(g):
            if not g.chunk_end:
                return
            p = g.h // 2
            hl = g.h % 2
            qrow = 64 * hl
            # raw yT drain (normalized later) + Z row -> zmat
            nc.vector.tensor_copy(
                yt[p][qrow:qrow + HD, 512 * g.c:512 * (g.c + 1)],
                g.py[0:64, :])
            i = 4 * hl + g.c
            zt = zt_p.tile([1, 512], F32, tag="zt", name="zt")
            nc.vector.tensor_copy(zt[:], g.py[64:65, :])
            nc.sync.dma_start(zmat[p][i:i + 1, :], zt[:])

        for p in range(NK):
            groups = make_groups(p)
            # fillers: norm for pair p-1 first, then qk projections for p+1
            if p > 0:
                filler_q.extend(norm_pieces(p - 1))
            if p + 1 < NK:
                for is_k in (False, True):
                    filler_q.extend(qk_pieces(is_k, p + 1))
            prev = None
            py_live = [None]
            for g in groups:
                emit_S(g)
                emit_EXP(g)
                # one filler piece per slot keeps PE ahead of ACT
                if filler_q:
                    filler_q.pop(0)()
                if prev is not None:
                    emit_AV(prev, py_live)
                    emit_post(prev)
                prev = g
            emit_AV(prev, py_live)
            emit_post(prev)

        # drain remaining fillers + last pair norm
        for pc in filler_q:
            pc()
        for pc in norm_pieces(NK - 1):
            pc()

        # ---------------- out projection ----------------
        for t in range(NT):
            po = sps()
            for oc in range(2):
                for k in range(NK):
                    nc.tensor.matmul(
                        po[:, 512 * oc:512 * (oc + 1)],
                        yt[k][:, 128 * t:128 * (t + 1)],
                        wo_sb[k][:, 512 * oc:512 * (oc + 1)],
                        start=(k == 0), stop=(k == NK - 1))
            ob = ob_p.tile([128, D], F32, tag="ob", name="ob")
            nc.vector.tensor_copy(ob[:], po[:])
            nc.sync.dma_start(out_d[128 * t:128 * (t + 1), :], ob[:])


def kernel(x, attn_mask, Wqkv, bqkv, Wout, bout):
    if "nc" not in _cached:
        _cached["nc"] = _build()
    nc = _cached["nc"]

    x = np.asarray(x, dtype=np.float32)
    Wqkv = np.asarray(Wqkv, dtype=np.float32)
    bqkv = np.asarray(bqkv, dtype=np.float32)
    Wout = np.asarray(Wout, dtype=np.float32)
    bout = np.asarray(bout, dtype=np.float32)
    # transposed causal diagonal block, bf16
    maskt_blk = np.ascontiguousarray(
        np.asarray(attn_mask, dtype=np.float32)[0, 0, :512, :512].T
    ).astype(bfloat16)

    in_maps = []
    for b in range(B):
        for g in range(G):
            s = slice(g * DG, (g + 1) * DG)
            in_maps.append({
                "x": np.ascontiguousarray(x[b]).astype(bfloat16),
                "wq": np.ascontiguousarray(Wqkv[:, g * DG:(g + 1) * DG]).astype(bfloat16),
                "wk": np.ascontiguousarray(Wqkv[:, D + g * DG:D + (g + 1) * DG]).astype(bfloat16),
                "wv": np.ascontiguousarray(Wqkv[:, 2 * D + g * DG:2 * D + (g + 1) * DG]).astype(bfloat16),
                "wo": np.ascontiguousarray(Wout[s, :]).astype(bfloat16),
                "bq": np.ascontiguousarray(bqkv[g * DG:(g + 1) * DG]),
                "bk": np.ascontiguousarray(bqkv[D + g * DG:D + (g + 1) * DG]),
                "bv": np.ascontiguousarray(bqkv[2 * D + g * DG:2 * D + (g + 1) * DG]).astype(bfloat16),
                "maskt": maskt_blk,
            })

    trace = bool(int(os.environ.get("BASS_ATTN_TRACE", "0")))
    res = bass_utils.run_bass_kernel_spmd(
        nc, in_maps, core_ids=list(range(B * G)), trace=trace)
    _cached["last_result"] = res

    out = np.empty((B, T, D), dtype=np.float32)
    for b in range(B):
        out[b] = res.results[2 * b]["out"] + res.results[2 * b + 1]["out"] \
            + bout[None, :]
    return out


# revision 19
# speedup vs baseline: 1.8778x; 1.2118x over previous
"""Causal self-attention (B=4, T=2048, D=1024, H=16, HD=64) on 8 TRN2 NeuronCores.

Sharding: core = (batch b in 0..3, head-group g in 0..1) -> data parallel on B,
tensor parallel over heads (8 heads per core). Each core computes a partial
out-projection for its head group; the host sums the pair of partials per batch
(plus bout) at unshard time.

Device kernel (per core), bf16 matmul pipeline with fp32 PSUM accumulate:
  1. xT via DMA-transpose (bf16 XBAR path)                     [128 x 2048] x8
  2. v  = x @ Wv (natural) + ones col interleaved per head     [2048, 8*65]
     qT/kT = (x @ Wq/Wk).T per head-pair, bias added on drain  [128, 2048] x8
  3. attention, software-pipelined in groups of 2 tk-tiles:
       S^T pair -> PSUM [128,1024] (mask accumulated via PE on diag tiles,
       strictly-upper columns trimmed), one batched EXP (ACT) per group into
       bf16, AV+Z fused matmuls (M=65, ones row = softmax denominator).
     ACT exp is the bottleneck engine; qk projections of the next head-pair
     and the 1/Z broadcast of the previous pair are chopped into small PE
     pieces and emitted between S(g) and AV(g-1) so ACT never starves.
  4. normalization per pair: batched reciprocal + esel-matmul broadcast of
     1/Z, yt scaled in place.
  5. out = yT.T @ Wout streamed PSUM -> DRAM (bout added on host).
"""

import os
import numpy as np
from ml_dtypes import bfloat16

import concourse.bass as bass
import concourse.tile as tile
from concourse import bacc, mybir
import concourse.bass_utils as bass_utils
from concourse.masks import make_identity

F32 = mybir.dt.float32
F32R = mybir.dt.float32r
BF16 = mybir.dt.bfloat16
AF = mybir.ActivationFunctionType
ALU = mybir.AluOpType

B, T, D, H = 4, 2048, 1024, 16
HD = D // H          # 64
G = 2                # head groups (TP degree)
HPG = H // G         # 8 heads per core
DG = HPG * HD        # 512 local qkv dims per core
NT = T // 128        # 16 row tiles
ND = D // 128        # 8 contraction tiles
NC = T // 512        # 4 tq chunks
NK = DG // 128       # 4 local-dim tiles (pairs)
VW = HD + 1          # 65: v columns per head incl. ones column

_cached = {}


def _build():
    nc = bacc.Bacc("TRN2", target_bir_lowering=False, debug=False, num_devices=8)

    x_d = nc.dram_tensor("x", [D, T], BF16, kind="ExternalInput")  # pre-transposed
    wq_d = nc.dram_tensor("wq", [D, DG], BF16, kind="ExternalInput")
    wk_d = nc.dram_tensor("wk", [D, DG], BF16, kind="ExternalInput")
    wv_d = nc.dram_tensor("wv", [D, DG], BF16, kind="ExternalInput")
    wo_d = nc.dram_tensor("wo", [DG, D], BF16, kind="ExternalInput")
    bq_d = nc.dram_tensor("bq", [DG], F32, kind="ExternalInput")
    bk_d = nc.dram_tensor("bk", [DG], F32, kind="ExternalInput")
    bv_d = nc.dram_tensor("bv", [DG], BF16, kind="ExternalInput")
    maskt_d = nc.dram_tensor("maskt", [512, 512], BF16, kind="ExternalInput")
    out_d = nc.dram_tensor("out", [T, D], F32, kind="ExternalOutput")

    with tile.TileContext(nc) as tc:
        with nc.allow_low_precision(reason="bf16 matmul pipeline, fp32 psum"):
            _emit(nc, tc, x_d, wq_d, wk_d, wv_d, wo_d, bq_d, bk_d, bv_d,
                  maskt_d, out_d)
    nc.finalize()
    return nc


def _emit(nc, tc, x_d, wq_d, wk_d, wv_d, wo_d, bq_d, bk_d, bv_d, maskt_d,
          out_d):
    from contextlib import ExitStack
    ctx = ExitStack()
    with ctx:
        # ---------------- pools ----------------
        const_p = ctx.enter_context(tc.tile_pool(name="const", bufs=1))
        xt_p = ctx.enter_context(tc.tile_pool(name="xt", bufs=1))
        qk_p = ctx.enter_context(tc.tile_pool(name="qk", bufs=1))
        vp_p = ctx.enter_context(tc.tile_pool(name="vp", bufs=1))
        yt_p = ctx.enter_context(tc.tile_pool(name="yt", bufs=1))
        w_p = ctx.enter_context(tc.tile_pool(name="w", bufs=1))
        wblk_p = ctx.enter_context(tc.tile_pool(name="wblk", bufs=4))
        pt_p = ctx.enter_context(tc.tile_pool(name="pt", bufs=3))
        zt_p = ctx.enter_context(tc.tile_pool(name="zt", bufs=3))
        ob_p = ctx.enter_context(tc.tile_pool(name="ob", bufs=3))
        s_ps = ctx.enter_context(tc.tile_pool(name="s_ps", bufs=3, space="PSUM"))
        y_ps = ctx.enter_context(tc.tile_pool(name="y_ps", bufs=2, space="PSUM"))

        def sps():
            return s_ps.tile([128, 1024], F32, tag="s", name="sgrp")

        # ---------------- constants ----------------
        identb = const_p.tile([128, 128], BF16, tag="identb")
        make_identity(nc, identb[:])

        # e0b: row 0 ones (bf16, bias broadcast)
        e0b = const_p.tile([128, 128], BF16, tag="e0b")
        nc.vector.memset(e0b[:], 0.0)
        nc.vector.memset(e0b[0:1, :], 1.0)

        # esel[p, 64*i + m] = (p == i) for i in 0..7 (1/Z broadcast select)
        e32 = const_p.tile([128, 512], F32, tag="e32")
        nc.gpsimd.memset(e32[:], 1.0)
        nc.gpsimd.affine_select(
            out=e32[:], in_=e32[:], compare_op=ALU.is_equal, fill=0.0,
            base=0, channel_multiplier=1, pattern=[[-1, 8], [0, 64]])
        esel = const_p.tile([128, 512], F32R, tag="esel")
        nc.vector.tensor_copy(esel[:], e32[:])

        # q/k bias columns [128, 8]: cols 0-3 = bq tiles, 4-7 = bk tiles
        bqk = const_p.tile([128, 8], F32, tag="bqk")
        nc.sync.dma_start(bqk[:, 0:NK], bq_d[:].rearrange("(f p) -> p f", p=128))
        nc.sync.dma_start(bqk[:, NK:2 * NK], bk_d[:].rearrange("(f p) -> p f", p=128))

        # bv broadcast [128, 512]
        bvrow = const_p.tile([128, DG], BF16, tag="bvrow")
        nc.vector.memset(bvrow[:], 0.0)
        nc.sync.dma_start(bvrow[0:1, :], bv_d[:].rearrange("(o n) -> o n", o=1))
        bv_bc = const_p.tile([128, DG], BF16, tag="bv_bc")
        pb = sps()
        nc.tensor.matmul(pb[:, 0:DG], e0b[:], bvrow[:], start=True, stop=True)
        nc.vector.tensor_copy(bv_bc[:], pb[:, 0:DG])

        # zmat (Z rows, one tile per pair) / zrec (1/Z, rows 0-7 live)
        zmat = [const_p.tile([8, 512], F32, tag=f"zmat{p}", name=f"zmat{p}")
                for p in range(NK)]
        zrec = const_p.tile([128, 512], F32R, tag="zrec")
        nc.vector.memset(zrec[:].bitcast(F32), 0.0)

        # transposed causal diag mask tiles (host pre-transposed)
        maskt = []
        for l in range(4):
            mt = const_p.tile([128, 512], BF16, tag=f"mt{l}", name=f"maskt{l}")
            nc.sync.dma_start(mt[:], maskt_d[128 * l:128 * (l + 1), :])
            maskt.append(mt)

        # ACT exp table preload (hide the ~2.7us table DMA in startup)
        dumm = const_p.tile([1, 16], F32, tag="dumm")
        nc.vector.memset(dumm[:], 0.0)
        nc.scalar.activation(dumm[:], dumm[:], AF.Exp, scale=1.0)

        # ---------------- xT (host pre-transposed, plain loads) ----------
        xt = [xt_p.tile([128, T], BF16, tag=f"xt{d}", name=f"xt{d}") for d in range(ND)]
        for d in range(ND):
            nc.sync.dma_start(xt[d][:], x_d[128 * d:128 * (d + 1), :])

        # ---------------- weights ----------------
        wv_sb = []
        for d in range(ND):
            wt = w_p.tile([128, DG], BF16, tag=f"wv{d}", name=f"wvt{d}")
            nc.sync.dma_start(wt[:], wv_d[128 * d:128 * (d + 1), :])
            wv_sb.append(wt)
        wo_sb = []
        for k in range(NK):
            wt = w_p.tile([128, D], BF16, tag=f"wo{k}", name=f"wot{k}")
            nc.sync.dma_start(wt[:], wo_d[128 * k:128 * (k + 1), :])
            wo_sb.append(wt)

        def load_wblk(is_k, f):
            src = wk_d if is_k else wq_d
            wblk = wblk_p.tile([128, D], BF16, tag="wblk")
            nc.sync.dma_start(
                wblk[:].rearrange("p (dt c) -> p dt c", dt=ND),
                src[:, 128 * f:128 * (f + 1)].rearrange("(dt p) c -> p dt c", p=128))
            return wblk

        # persistent SBUF tensors
        vp = []
        for t in range(NT):
            vt = vp_p.tile([128, HPG * VW], BF16, tag=f"vp{t}", name=f"vp{t}")
            nc.vector.memset(vt[:], 1.0)
            vp.append(vt)
        qk = [qk_p.tile([128, T], BF16, tag=f"qk{f}", name=f"qk{f}")
              for f in range(2 * NK)]
        yt = [yt_p.tile([128, T], BF16, tag=f"yt{k}", name=f"yt{k}")
              for k in range(NK)]

        # ---------------- v projection (plain, upfront) ----------------
        for t in range(NT):
            pv = sps()
            for d in range(ND):
                nc.tensor.matmul(pv[:, 0:DG], xt[d][:, 128 * t:128 * (t + 1)],
                                 wv_sb[d][:], start=(d == 0), stop=(d == ND - 1))
            nc.vector.tensor_tensor(
                vp[t][:].rearrange("p (h c) -> p h c", h=HPG)[:, :, 0:HD],
                pv[:, 0:DG].rearrange("p (h c) -> p h c", h=HPG),
                bv_bc[:].rearrange("p (h c) -> p h c", h=HPG),
                ALU.add)

        # qk projection piece generator: one f-block = 4 chunks x 8 d-matmuls,
        # yielded in 8 pieces of 4 matmuls (chunk halves), drain per chunk.
        # The psum tile is allocated when the first half RUNS (not at
        # generator-build time) so pool rotation order matches emission order.
        def qk_pieces(is_k, f):
            wblk = load_wblk(is_k, f)
            dst = qk[NK + f if is_k else f]
            bcol = NK + f if is_k else f
            state = {}
            for cidx in range(NC):
                for half in range(2):
                    def piece(wblk=wblk, cidx=cidx, half=half,
                              dst=dst, bcol=bcol):
                        if half == 0:
                            state[cidx] = sps()
                        pq = state[cidx]
                        for d in range(4 * half, 4 * half + 4):
                            nc.tensor.matmul(
                                pq[:, 0:512], wblk[:, 128 * d:128 * (d + 1)],
                                xt[d][:, 512 * cidx:512 * (cidx + 1)],
                                start=(d == 0), stop=(d == ND - 1))
                        if half == 1:
                            nc.vector.tensor_scalar(
                                dst[:, 512 * cidx:512 * (cidx + 1)],
                                pq[:, 0:512], bqk[:, bcol:bcol + 1], None,
                                ALU.add)
                    yield piece

        # qk pair 0 upfront (plain)
        for is_k in (False, True):
            for pc in qk_pieces(is_k, 0):
                pc()

        # ---------------- attention ----------------
        # per-pair normalization pieces (reciprocal batched per pair,
        # broadcast via esel matmul, yt scaled in place)
        def norm_pieces(p):
            def recip(p=p):
                nc.vector.reciprocal(zrec[0:8, :], zmat[p][0:8, :])
            yield recip
            for hl in range(2):
                for c in range(NC):
                    def piece(p=p, hl=hl, c=c):
                        i = 4 * hl + c
                        qrow = 64 * hl
                        zb = sps()
                        nc.tensor.matmul(zb[0:64, 0:512],
                                         esel[:, 64 * i:64 * (i + 1)],
                                         zrec[:],
                                         start=True, stop=True)
                        ysl = yt[p][qrow:qrow + HD, 512 * c:512 * (c + 1)]
                        nc.vector.tensor_tensor(ysl, ysl, zb[0:64, 0:512],
                                                ALU.mult)
                    yield piece

        filler_q = []  # queued PE filler pieces

        class Grp:
            __slots__ = ("p", "c", "j", "d", "off", "ps", "pt", "py",
                         "chunk_end")

        def make_groups(p):
            gs = []
            for c in range(NC):
                for j in range(4 * c + 4):
                    g = Grp()
                    g.p, g.c, g.j = p, c, j
                    g.d = j - 4 * c
                    g.off = 128 * g.d if g.d >= 0 else 0
                    g.chunk_end = (j == 4 * c + 3)
                    gs.append(g)
            return gs

        def emit_S(g):
            # head pair: even head at PE rows 0-63, odd at 64-127 -> the two
            # S matmuls land in distinct row groups and psum banks and run
            # concurrently (auto tile_position from base partitions).
            qt, kt = qk[g.p], qk[NK + g.p]
            g.ps = sps()
            for hl in range(2):
                qrow = 64 * hl
                nc.tensor.matmul(
                    g.ps[:, 512 * hl + g.off:512 * (hl + 1)],
                    kt[qrow:qrow + HD, 128 * g.j:128 * (g.j + 1)],
                    qt[qrow:qrow + HD, 512 * g.c + g.off:512 * (g.c + 1)],
                    start=True, stop=(g.d < 0), skip_group_check=True)
            if g.d >= 0:
                for hl in range(2):
                    nc.tensor.matmul(
                        g.ps[:, 512 * hl + g.off:512 * (hl + 1)], identb[:],
                        maskt[g.d][:, g.off:512],
                        start=False, stop=True, skip_group_check=True)

        def emit_EXP(g):
            g.pt = pt_p.tile([128, 1024], BF16, tag="pt", name="pt")
            if g.off:
                psv = g.ps[:].rearrange("p (two n) -> p two n", two=2)
                ptv = g.pt[:].rearrange("p (two n) -> p two n", two=2)
                nc.scalar.activation(ptv[:, :, g.off:512], psv[:, :, g.off:512],
                                     AF.Exp, scale=0.125)
            else:
                nc.scalar.activation(g.pt[:], g.ps[:], AF.Exp, scale=0.125)

        def emit_AV(g, py_live):
            if g.j == 0:
                py_live[0] = y_ps.tile([VW, 512], F32, tag="py", name="py0")
                py_live[1] = y_ps.tile([VW, 512], F32, tag="py", name="py1")
            g.py = (py_live[0], py_live[1])
            jmax = 4 * g.c + 3
            for hl in range(2):
                hidx = (2 * g.p + hl) % HPG
                nc.tensor.matmul(
                    g.py[hl][:, g.off:512],
                    vp[g.j][:, VW * hidx:VW * hidx + VW],
                    g.pt[:, 512 * hl + g.off:512 * (hl + 1)],
                    start=(g.j == 0), stop=(g.j == jmax),
                    skip_group_check=True)

        def emit_post(g):
            if not g.chunk_end:
                return
            for hl in range(2):
                qrow = 64 * hl
                # raw yT drain (normalized later) + Z row -> zmat
                nc.vector.tensor_copy(
                    yt[g.p][qrow:qrow + HD, 512 * g.c:512 * (g.c + 1)],
                    g.py[hl][0:64, :])
                i = 4 * hl + g.c
                zt = zt_p.tile([1, 512], F32, tag="zt", name="zt")
                nc.vector.tensor_copy(zt[:], g.py[hl][64:65, :])
                nc.sync.dma_start(zmat[g.p][i:i + 1, :], zt[:])

        for p in range(NK):
            groups = make_groups(p)
            # fillers: norm for pair p-1 first, then qk projections for p+1
            if p > 0:
                filler_q.extend(norm_pieces(p - 1))
            if p + 1 < NK:
                for is_k in (False, True):
                    filler_q.extend(qk_pieces(is_k, p + 1))
            prev = None
            py_live = [None, None]
            for g in groups:
                emit_S(g)
                emit_EXP(g)
                # one filler piece per slot keeps PE ahead of ACT
                if filler_q:
                    filler_q.pop(0)()
                if prev is not None:
                    emit_AV(prev, py_live)
                    emit_post(prev)
                prev = g
            emit_AV(prev, py_live)
            emit_post(prev)

        # drain remaining fillers + last pair norm
        for pc in filler_q:
            pc()
        for pc in norm_pieces(NK - 1):
            pc()

        # ---------------- out projection ----------------
        for t in range(NT):
            po = sps()
            for oc in range(2):
                for k in range(NK):
                    nc.tensor.matmul(
                        po[:, 512 * oc:512 * (oc + 1)],
                        yt[k][:, 128 * t:128 * (t + 1)],
                        wo_sb[k][:, 512 * oc:512 * (oc + 1)],
                        start=(k == 0), stop=(k == NK - 1))
            ob = ob_p.tile([128, D], F32, tag="ob", name="ob")
            nc.vector.tensor_copy(ob[:, 0:512], po[:, 0:512])
            nc.scalar.copy(ob[:, 512:1024], po[:, 512:1024])
            nc.sync.dma_start(out_d[128 * t:128 * (t + 1), :], ob[:])


def kernel(x, attn_mask, Wqkv, bqkv, Wout, bout):
    if "nc" not in _cached:
        _cached["nc"] = _build()
    nc = _cached["nc"]

    x = np.asarray(x, dtype=np.float32)
    Wqkv = np.asarray(Wqkv, dtype=np.float32)
    bqkv = np.asarray(bqkv, dtype=np.float32)
    Wout = np.asarray(Wout, dtype=np.float32)
    bout = np.asarray(bout, dtype=np.float32)
    # transposed causal diagonal block, bf16
    maskt_blk = np.ascontiguousarray(
        np.asarray(attn_mask, dtype=np.float32)[0, 0, :512, :512].T
    ).astype(bfloat16)

    in_maps = []
    for b in range(B):
        for g in range(G):
            s = slice(g * DG, (g + 1) * DG)
            in_maps.append({
                "x": np.ascontiguousarray(x[b].T).astype(bfloat16),
                "wq": np.ascontiguousarray(Wqkv[:, g * DG:(g + 1) * DG]).astype(bfloat16),
                "wk": np.ascontiguousarray(Wqkv[:, D + g * DG:D + (g + 1) * DG]).astype(bfloat16),
                "wv": np.ascontiguousarray(Wqkv[:, 2 * D + g * DG:2 * D + (g + 1) * DG]).astype(bfloat16),
                "wo": np.ascontiguousarray(Wout[s, :]).astype(bfloat16),
                "bq": np.ascontiguousarray(bqkv[g * DG:(g + 1) * DG]),
                "bk": np.ascontiguousarray(bqkv[D + g * DG:D + (g + 1) * DG]),
                "bv": np.ascontiguousarray(bqkv[2 * D + g * DG:2 * D + (g + 1) * DG]).astype(bfloat16),
                "maskt": maskt_blk,
            })

    trace = bool(int(os.environ.get("BASS_ATTN_TRACE", "0")))
    res = bass_utils.run_bass_kernel_spmd(
        nc, in_maps, core_ids=list(range(B * G)), trace=trace)
    _cached["last_result"] = res

    out = np.empty((B, T, D), dtype=np.float32)
    for b in range(B):
        out[b] = res.results[2 * b]["out"] + res.results[2 * b + 1]["out"] \
            + bout[None, :]
    return out


# revision 27
# speedup vs baseline: 1.9316x; 1.0287x over previous
"""Causal self-attention (B=4, T=2048, D=1024, H=16, HD=64) on 8 TRN2 NeuronCores.

Sharding: core = (batch b in 0..3, head-group g in 0..1) -> data parallel on B,
tensor parallel over heads (8 heads per core). Each core computes a partial
out-projection for its head group; the host sums the pair of partials per batch
(plus bout) at unshard time.

Device kernel (per core), bf16 matmul pipeline with fp32 PSUM accumulate:
  1. xT via DMA-transpose (bf16 XBAR path)                     [128 x 2048] x8
  2. v  = x @ Wv (natural) + ones col interleaved per head     [2048, 8*65]
     qT/kT = (x @ Wq/Wk).T per head-pair, bias added on drain  [128, 2048] x8
  3. attention, software-pipelined in groups of 2 tk-tiles:
       S^T pair -> PSUM [128,1024] (mask accumulated via PE on diag tiles,
       strictly-upper columns trimmed), one batched EXP (ACT) per group into
       bf16, AV+Z fused matmuls (M=65, ones row = softmax denominator).
     ACT exp is the bottleneck engine; qk projections of the next head-pair
     and the 1/Z broadcast of the previous pair are chopped into small PE
     pieces and emitted between S(g) and AV(g-1) so ACT never starves.
  4. normalization per pair: batched reciprocal + esel-matmul broadcast of
     1/Z, yt scaled in place.
  5. out = yT.T @ Wout streamed PSUM -> DRAM (bout added on host).
"""

import os
import numpy as np
from ml_dtypes import bfloat16

import concourse.bass as bass
import concourse.tile as tile
from concourse import bacc, mybir
import concourse.bass_utils as bass_utils
from concourse.masks import make_identity

F32 = mybir.dt.float32
F32R = mybir.dt.float32r
BF16 = mybir.dt.bfloat16
AF = mybir.ActivationFunctionType
ALU = mybir.AluOpType

B, T, D, H = 4, 2048, 1024, 16
HD = D // H          # 64
G = 2                # head groups (TP degree)
HPG = H // G         # 8 heads per core
DG = HPG * HD        # 512 local qkv dims per core
NT = T // 128        # 16 row tiles
ND = D // 128        # 8 contraction tiles
NC = T // 512        # 4 tq chunks
NK = DG // 128       # 4 local-dim tiles (pairs)
VW = HD + 1          # 65: v columns per head incl. ones column

_cached = {}


def _build():
    nc = bacc.Bacc("TRN2", target_bir_lowering=False, debug=False, num_devices=8)

    x_d = nc.dram_tensor("x", [D, T], BF16, kind="ExternalInput")  # pre-transposed
    wq_d = nc.dram_tensor("wq", [D, DG], BF16, kind="ExternalInput")
    wk_d = nc.dram_tensor("wk", [D, DG], BF16, kind="ExternalInput")
    wv_d = nc.dram_tensor("wv", [D, DG], BF16, kind="ExternalInput")
    wo_d = nc.dram_tensor("wo", [DG, D], BF16, kind="ExternalInput")
    bq_d = nc.dram_tensor("bq", [DG], F32, kind="ExternalInput")
    bk_d = nc.dram_tensor("bk", [DG], F32, kind="ExternalInput")
    bv_d = nc.dram_tensor("bv", [DG], BF16, kind="ExternalInput")
    maskt_d = nc.dram_tensor("maskt", [512, 512], BF16, kind="ExternalInput")
    out_d = nc.dram_tensor("out", [T, D], F32, kind="ExternalOutput")

    with tile.TileContext(nc) as tc:
        with nc.allow_low_precision(reason="bf16 matmul pipeline, fp32 psum"):
            _emit(nc, tc, x_d, wq_d, wk_d, wv_d, wo_d, bq_d, bk_d, bv_d,
                  maskt_d, out_d)
    nc.finalize()
    return nc


def _emit(nc, tc, x_d, wq_d, wk_d, wv_d, wo_d, bq_d, bk_d, bv_d, maskt_d,
          out_d):
    from contextlib import ExitStack
    ctx = ExitStack()
    with ctx:
        # ---------------- pools ----------------
        const_p = ctx.enter_context(tc.tile_pool(name="const", bufs=1))
        xt_p = ctx.enter_context(tc.tile_pool(name="xt", bufs=1))
        qk_p = ctx.enter_context(tc.tile_pool(name="qk", bufs=1))
        vp_p = ctx.enter_context(tc.tile_pool(name="vp", bufs=1))
        yt_p = ctx.enter_context(tc.tile_pool(name="yt", bufs=1))
        w_p = ctx.enter_context(tc.tile_pool(name="w", bufs=1))
        wblk_p = ctx.enter_context(tc.tile_pool(name="wblk", bufs=4))
        pt_p = ctx.enter_context(tc.tile_pool(name="pt", bufs=3))
        zt_p = ctx.enter_context(tc.tile_pool(name="zt", bufs=3))
        ob_p = ctx.enter_context(tc.tile_pool(name="ob", bufs=3))
        s_ps = ctx.enter_context(tc.tile_pool(name="s_ps", bufs=3, space="PSUM"))
        y_ps = ctx.enter_context(tc.tile_pool(name="y_ps", bufs=2, space="PSUM"))

        def sps():
            return s_ps.tile([128, 1024], F32, tag="s", name="sgrp")

        # ---------------- constants ----------------
        identb = const_p.tile([128, 128], BF16, tag="identb")
        make_identity(nc, identb[:])

        # e0b: row 0 ones (bf16, bias broadcast)
        e0b = const_p.tile([128, 128], BF16, tag="e0b")
        nc.vector.memset(e0b[:], 0.0)
        nc.vector.memset(e0b[0:1, :], 1.0)

        # esel_c[r, m] = (r == 4*(m//64) + c): selects zrec rows (c, 4+c) into
        # the two 64-row halves -> one matmul broadcasts 1/Z for both heads.
        esel = []
        for c in range(NC):
            et = const_p.tile([128, 128], BF16, tag=f"esel{c}", name=f"esel{c}")
            nc.gpsimd.memset(et[:], 1.0)
            nc.gpsimd.affine_select(
                out=et[:], in_=et[:], compare_op=ALU.is_equal, fill=0.0,
                base=-c, channel_multiplier=1, pattern=[[-4, 2], [0, 64]])
            esel.append(et)

        # q/k bias columns [128, 8]: cols 0-3 = bq tiles, 4-7 = bk tiles
        bqk = const_p.tile([128, 8], F32, tag="bqk")
        nc.sync.dma_start(bqk[:, 0:NK], bq_d[:].rearrange("(f p) -> p f", p=128))
        nc.sync.dma_start(bqk[:, NK:2 * NK], bk_d[:].rearrange("(f p) -> p f", p=128))

        # bv broadcast [128, 512]
        bvrow = const_p.tile([128, DG], BF16, tag="bvrow")
        nc.vector.memset(bvrow[:], 0.0)
        nc.sync.dma_start(bvrow[0:1, :], bv_d[:].rearrange("(o n) -> o n", o=1))
        bv_bc = const_p.tile([128, DG], BF16, tag="bv_bc")
        pb = sps()
        nc.tensor.matmul(pb[:, 0:DG], e0b[:], bvrow[:], start=True, stop=True)
        nc.vector.tensor_copy(bv_bc[:], pb[:, 0:DG])

        # zmat (Z rows, one tile per pair) / zrec (1/Z bf16, rows 0-7 live)
        zmat = [const_p.tile([8, 512], F32, tag=f"zmat{p}", name=f"zmat{p}")
                for p in range(NK)]
        zrec = const_p.tile([128, 512], BF16, tag="zrec")
        nc.vector.memset(zrec[:], 0.0)

        # transposed causal diag mask tiles (host pre-transposed)
        maskt = []
        for l in range(4):
            mt = const_p.tile([128, 512], BF16, tag=f"mt{l}", name=f"maskt{l}")
            nc.sync.dma_start(mt[:], maskt_d[128 * l:128 * (l + 1), :])
            maskt.append(mt)

        # ACT exp table preload (hide the ~2.7us table DMA in startup)
        dumm = const_p.tile([1, 16], F32, tag="dumm")
        nc.vector.memset(dumm[:], 0.0)
        nc.scalar.activation(dumm[:], dumm[:], AF.Exp, scale=1.0)

        # ---------------- xT (host pre-transposed, plain loads) ----------
        xt = [xt_p.tile([128, T], BF16, tag=f"xt{d}", name=f"xt{d}") for d in range(ND)]
        for d in range(ND):
            nc.sync.dma_start(xt[d][:], x_d[128 * d:128 * (d + 1), :])

        # ---------------- weights ----------------
        wv_sb = []
        for d in range(ND):
            wt = w_p.tile([128, DG], BF16, tag=f"wv{d}", name=f"wvt{d}")
            nc.sync.dma_start(wt[:], wv_d[128 * d:128 * (d + 1), :])
            wv_sb.append(wt)
        wo_sb = []
        for k in range(NK):
            wt = w_p.tile([128, D], BF16, tag=f"wo{k}", name=f"wot{k}")
            nc.sync.dma_start(wt[:], wo_d[128 * k:128 * (k + 1), :])
            wo_sb.append(wt)

        def load_wblk(is_k, f):
            src = wk_d if is_k else wq_d
            wblk = wblk_p.tile([128, D], BF16, tag="wblk")
            nc.sync.dma_start(
                wblk[:].rearrange("p (dt c) -> p dt c", dt=ND),
                src[:, 128 * f:128 * (f + 1)].rearrange("(dt p) c -> p dt c", p=128))
            return wblk

        # persistent SBUF tensors
        vp = []
        for t in range(NT):
            vt = vp_p.tile([128, HPG * VW], BF16, tag=f"vp{t}", name=f"vp{t}")
            nc.vector.memset(vt[:], 1.0)
            vp.append(vt)
        qk = [qk_p.tile([128, T], BF16, tag=f"qk{f}", name=f"qk{f}")
              for f in range(2 * NK)]
        yt = [yt_p.tile([128, T], BF16, tag=f"yt{k}", name=f"yt{k}")
              for k in range(NK)]

        # ---------------- v projection (plain, upfront) ----------------
        for t in range(NT):
            pv = sps()
            for d in range(ND):
                nc.tensor.matmul(pv[:, 0:DG], xt[d][:, 128 * t:128 * (t + 1)],
                                 wv_sb[d][:], start=(d == 0), stop=(d == ND - 1))
            nc.vector.tensor_tensor(
                vp[t][:].rearrange("p (h c) -> p h c", h=HPG)[:, :, 0:HD],
                pv[:, 0:DG].rearrange("p (h c) -> p h c", h=HPG),
                bv_bc[:].rearrange("p (h c) -> p h c", h=HPG),
                ALU.add)

        # qk projection piece generator: one f-block = 4 chunks x 8 d-matmuls,
        # yielded in 8 pieces of 4 matmuls (chunk halves), drain per chunk.
        # The psum tile is allocated when the first half RUNS (not at
        # generator-build time) so pool rotation order matches emission order.
        def qk_pieces(is_k, f):
            wblk = load_wblk(is_k, f)
            dst = qk[NK + f if is_k else f]
            bcol = NK + f if is_k else f
            state = {}
            for cidx in range(NC):
                for half in range(2):
                    def piece(wblk=wblk, cidx=cidx, half=half,
                              dst=dst, bcol=bcol):
                        if half == 0:
                            state[cidx] = sps()
                        pq = state[cidx]
                        for d in range(4 * half, 4 * half + 4):
                            nc.tensor.matmul(
                                pq[:, 0:512], wblk[:, 128 * d:128 * (d + 1)],
                                xt[d][:, 512 * cidx:512 * (cidx + 1)],
                                start=(d == 0), stop=(d == ND - 1))
                        if half == 1:
                            nc.vector.tensor_scalar(
                                dst[:, 512 * cidx:512 * (cidx + 1)],
                                pq[:, 0:512], bqk[:, bcol:bcol + 1], None,
                                ALU.add)
                    yield piece

        # qk pair 0 upfront (plain)
        for is_k in (False, True):
            for pc in qk_pieces(is_k, 0):
                pc()

        # ---------------- attention ----------------
        # per-pair normalization pieces (reciprocal batched per pair,
        # broadcast via esel matmul, yt scaled in place)
        def norm_pieces(p):
            def recip(p=p):
                nc.vector.reciprocal(zrec[0:8, :], zmat[p][0:8, :])
            yield recip
            for c in range(NC):
                def piece(p=p, c=c):
                    zb = sps()
                    nc.tensor.matmul(zb[:, 0:512], esel[c][:], zrec[:],
                                     start=True, stop=True)
                    ysl = yt[p][:, 512 * c:512 * (c + 1)]
                    nc.vector.tensor_tensor(ysl, ysl, zb[:, 0:512], ALU.mult)
                yield piece

        filler_q = []  # queued PE filler pieces

        class Grp:
            __slots__ = ("p", "c", "j", "d", "off", "ps", "pt", "py",
                         "chunk_end")

        def make_groups(p):
            gs = []
            for c in range(NC):
                for j in range(4 * c + 4):
                    g = Grp()
                    g.p, g.c, g.j = p, c, j
                    g.d = j - 4 * c
                    g.off = 128 * g.d if g.d >= 0 else 0
                    g.chunk_end = (j == 4 * c + 3)
                    gs.append(g)
            return gs

        def emit_S(g):
            # head pair: even head at PE rows 0-63, odd at 64-127 -> the two
            # S matmuls land in distinct row groups and psum banks and run
            # concurrently (auto tile_position from base partitions).
            qt, kt = qk[g.p], qk[NK + g.p]
            g.ps = sps()
            for hl in range(2):
                qrow = 64 * hl
                nc.tensor.matmul(
                    g.ps[:, 512 * hl + g.off:512 * (hl + 1)],
                    kt[qrow:qrow + HD, 128 * g.j:128 * (g.j + 1)],
                    qt[qrow:qrow + HD, 512 * g.c + g.off:512 * (g.c + 1)],
                    start=True, stop=(g.d < 0), skip_group_check=True)
            if g.d >= 0:
                for hl in range(2):
                    nc.tensor.matmul(
                        g.ps[:, 512 * hl + g.off:512 * (hl + 1)], identb[:],
                        maskt[g.d][:, g.off:512],
                        start=False, stop=True, skip_group_check=True)

        def emit_EXP(g):
            g.pt = pt_p.tile([128, 1024], BF16, tag="pt", name="pt")
            if g.off:
                psv = g.ps[:].rearrange("p (two n) -> p two n", two=2)
                ptv = g.pt[:].rearrange("p (two n) -> p two n", two=2)
                nc.scalar.activation(ptv[:, :, g.off:512], psv[:, :, g.off:512],
                                     AF.Exp, scale=0.125)
            else:
                nc.scalar.activation(g.pt[:], g.ps[:], AF.Exp, scale=0.125)

        def emit_AV(g, py_live):
            if g.j == 0:
                py_live[0] = y_ps.tile([VW, 512], F32, tag="py", name="py0")
                py_live[1] = y_ps.tile([VW, 512], F32, tag="py", name="py1")
            g.py = (py_live[0], py_live[1])
            jmax = 4 * g.c + 3
            for hl in range(2):
                hidx = (2 * g.p + hl) % HPG
                nc.tensor.matmul(
                    g.py[hl][:, g.off:512],
                    vp[g.j][:, VW * hidx:VW * hidx + VW],
                    g.pt[:, 512 * hl + g.off:512 * (hl + 1)],
                    start=(g.j == 0), stop=(g.j == jmax),
                    skip_group_check=True)

        def emit_post(g):
            if not g.chunk_end:
                return
            for hl in range(2):
                qrow = 64 * hl
                # raw yT drain (normalized later) + Z row -> zmat
                nc.vector.tensor_copy(
                    yt[g.p][qrow:qrow + HD, 512 * g.c:512 * (g.c + 1)],
                    g.py[hl][0:64, :])
                i = 4 * hl + g.c
                zt = zt_p.tile([1, 512], F32, tag="zt", name="zt")
                nc.vector.tensor_copy(zt[:], g.py[hl][64:65, :])
                nc.sync.dma_start(zmat[g.p][i:i + 1, :], zt[:])

        for p in range(NK):
            groups = make_groups(p)
            # fillers: norm for pair p-1 first, then qk projections for p+1
            if p > 0:
                filler_q.extend(norm_pieces(p - 1))
            if p + 1 < NK:
                for is_k in (False, True):
                    filler_q.extend(qk_pieces(is_k, p + 1))
            prev = None
            py_live = [None, None]
            for g in groups:
                emit_S(g)
                emit_EXP(g)
                if prev is not None:
                    emit_AV(prev, py_live)
                    emit_post(prev)
                # one filler piece per slot keeps PE ahead of ACT
                if filler_q:
                    filler_q.pop(0)()
                prev = g
            emit_AV(prev, py_live)
            emit_post(prev)

        # drain remaining fillers + last pair norm
        for pc in filler_q:
            pc()
        for pc in norm_pieces(NK - 1):
            pc()

        # ---------------- out projection ----------------
        for t in range(NT):
            po = sps()
            for oc in range(2):
                for k in range(NK):
                    nc.tensor.matmul(
                        po[:, 512 * oc:512 * (oc + 1)],
                        yt[k][:, 128 * t:128 * (t + 1)],
                        wo_sb[k][:, 512 * oc:512 * (oc + 1)],
                        start=(k == 0), stop=(k == NK - 1))
            ob = ob_p.tile([128, D], F32, tag="ob", name="ob")
            nc.vector.tensor_copy(ob[:, 0:512], po[:, 0:512])
            nc.scalar.copy(ob[:, 512:1024], po[:, 512:1024])
            nc.sync.dma_start(out_d[128 * t:128 * (t + 1), :], ob[:])


def kernel(x, attn_mask, Wqkv, bqkv, Wout, bout):
    if "nc" not in _cached:
        _cached["nc"] = _build()
    nc = _cached["nc"]

    x = np.asarray(x, dtype=np.float32)
    Wqkv = np.asarray(Wqkv, dtype=np.float32)
    bqkv = np.asarray(bqkv, dtype=np.float32)
    Wout = np.asarray(Wout, dtype=np.float32)
    bout = np.asarray(bout, dtype=np.float32)
    # transposed causal diagonal block, bf16
    maskt_blk = np.ascontiguousarray(
        np.asarray(attn_mask, dtype=np.float32)[0, 0, :512, :512].T
    ).astype(bfloat16)

    in_maps = []
    for b in range(B):
        for g in range(G):
            s = slice(g * DG, (g + 1) * DG)
            in_maps.append({
                "x": np.ascontiguousarray(x[b].T).astype(bfloat16),
                "wq": np.ascontiguousarray(Wqkv[:, g * DG:(g + 1) * DG]).astype(bfloat16),
                "wk": np.ascontiguousarray(Wqkv[:, D + g * DG:D + (g + 1) * DG]).astype(bfloat16),
                "wv": np.ascontiguousarray(Wqkv[:, 2 * D + g * DG:2 * D + (g + 1) * DG]).astype(bfloat16),
                "wo": np.ascontiguousarray(Wout[s, :]).astype(bfloat16),
                "bq": np.ascontiguousarray(bqkv[g * DG:(g + 1) * DG]),
                "bk": np.ascontiguousarray(bqkv[D + g * DG:D + (g + 1) * DG]),
                "bv": np.ascontiguousarray(bqkv[2 * D + g * DG:2 * D + (g + 1) * DG]).astype(bfloat16),
                "maskt": maskt_blk,
            })

    trace = bool(int(os.environ.get("BASS_ATTN_TRACE", "0")))
    res = bass_utils.run_bass_kernel_spmd(
        nc, in_maps, core_ids=list(range(B * G)), trace=trace)
    _cached["last_result"] = res

    out = np.empty((B, T, D), dtype=np.float32)
    for b in range(B):
        out[b] = res.results[2 * b]["out"] + res.results[2 * b + 1]["out"] \
            + bout[None, :]
    return out
